# revision 7
# baseline (speedup 1.0000x reference)
"""Trainium2 Bass kernel for an AttnBlock++ (GroupNorm -> QKV 1x1 conv ->
full softmax attention over HW tokens -> output projection -> residual/sqrt(2)).

Sharding: data-parallel over batch B=8 across the 8 NeuronCores; attention is
fully independent per batch element, so each core processes one [C, H*W]
feature map with no collectives.

Per-core algorithm (C=256, N=H*W=4096, 32 groups):
  - GroupNorm is folded into the QKV weights: hn = fs*x + fb (per-channel
    affine from group stats), so q/k/v = (W*diag(fs)) @ x + const. The K-side
    constant drops out of softmax exactly (shift invariance); the V-side
    constant is folded into the output-projection bias (attention rows sum
    to 1); only the Q-side constant is applied.
  - Scores are computed transposed (S_T[m, n], keys on partitions) so no
    transposes are ever needed: exp on the scalar engine, softmax denominator
    via a ones-vector matmul on the tensor engine (partition-axis sum), and
    att@V with natural [c, n] output. Division by the denominator commutes
    with the output projection and is applied at the very end.
  - 1/sqrt(2) of the residual is folded into the output weights and biases.
"""

import math

import numpy as np

import concourse.bacc as bacc
import concourse.tile as tile
from concourse import mybir
from concourse import bass_utils

B, C, H, W = 8, 256, 64, 64
N = H * W  # 4096
G = 32  # groups
GD = C // G  # 8 channels per group
EPS = 1e-6
NCORES = 8
NCH = 2  # channel chunks of 128
NBLK = 8  # query blocks
BLK = 512  # queries per block
MT = 32  # key tiles of 128
SCALE = float(C) ** -0.5  # 1/16
INV_SQRT2 = float(1.0 / math.sqrt(2.0))
NWARM = 56  # HAM warm-up matmuls issued while phase 0 runs

FP32 = mybir.dt.float32
BF16 = mybir.dt.bfloat16
AF = mybir.ActivationFunctionType
ALU = mybir.AluOpType


def build_program():
    nc = bacc.Bacc("TRN2", target_bir_lowering=False, debug=False)

    x = nc.dram_tensor("x", [C, N], FP32, kind="ExternalInput").ap()
    wqT = nc.dram_tensor("wqT", [C, C], FP32, kind="ExternalInput").ap()
    wkT = nc.dram_tensor("wkT", [C, C], FP32, kind="ExternalInput").ap()
    wvT = nc.dram_tensor("wvT", [C, C], FP32, kind="ExternalInput").ap()
    woT = nc.dram_tensor("woT", [C, C], FP32, kind="ExternalInput").ap()
    bq = nc.dram_tensor("bq", [C, 1], FP32, kind="ExternalInput").ap()
    bv = nc.dram_tensor("bv", [C, 1], FP32, kind="ExternalInput").ap()
    bo = nc.dram_tensor("bo", [C, 1], FP32, kind="ExternalInput").ap()
    gns = nc.dram_tensor("gns", [C, 1], FP32, kind="ExternalInput").ap()
    gnb = nc.dram_tensor("gnb", [C, 1], FP32, kind="ExternalInput").ap()
    # ind16[c, g] = 1/8 if c//8 == g else 0 (group-average over channels)
    ind16 = nc.dram_tensor("ind16", [128, 16], FP32, kind="ExternalInput").ap()
    # bcast16[g, c] = 1 if c//8 == g else 0 (broadcast group value to channels)
    bcast16 = nc.dram_tensor("bcast16", [16, 128], FP32, kind="ExternalInput").ap()
    y = nc.dram_tensor("y", [C, N], FP32, kind="ExternalOutput").ap()

    with tile.TileContext(nc) as tc:
        with (
            tc.tile_pool(name="persist", bufs=1) as P,
            tc.tile_pool(name="work", bufs=3) as WK,
        ):
            # ---------------- constants + HAM warm-up ----------------
            junk_bf = P.tile([128, BLK], BF16, tag="junk")
            nc.gpsimd.memset(junk_bf, 0.0)
            ones128_bf = P.tile([128, 1], BF16, tag="ones128")
            nc.vector.memset(ones128_bf, 1.0)
            ones1_f32 = P.tile([1, 128], FP32, tag="ones1")
            nc.vector.memset(ones1_f32, 1.0)
            eps16 = P.tile([16, 1], FP32, tag="eps16")
            nc.vector.memset(eps16, EPS)

            # ---------------- load inputs (x first: stats critical path) ----
            x_sb = []
            for j in range(NCH):
                t = P.tile([128, N], FP32, tag=f"x{j}", name=f"x{j}")
                x_sb.append(t)
            for j in range(NCH):
                for p in range(4):
                    cs = slice(p * 1024, (p + 1) * 1024)
                    nc.sync.dma_start(
                        out=x_sb[j][:, cs], in_=x[j * 128:(j + 1) * 128, cs]
                    )

            def load2(ap, name, width):
                ts = []
                for j in range(NCH):
                    t = P.tile([128, width], FP32, tag=f"{name}{j}", name=f"{name}{j}")
                    nc.sync.dma_start(out=t, in_=ap[j * 128:(j + 1) * 128, :])
                    ts.append(t)
                return ts

            ind16_sb = P.tile([128, 16], FP32, tag="ind16")
            nc.sync.dma_start(out=ind16_sb, in_=ind16)
            bcast16_sb = P.tile([16, 128], FP32, tag="bcast16")
            nc.sync.dma_start(out=bcast16_sb, in_=bcast16)
            gns_sb = load2(gns, "gns", 1)
            gnb_sb = load2(gnb, "gnb", 1)
            bq_sb = load2(bq, "bq", 1)
            bv_sb = load2(bv, "bv", 1)
            bo_sb = load2(bo, "bo", 1)
            wqT_sb = load2(wqT, "wqT", C)
            wkT_sb = load2(wkT, "wkT", C)
            wvT_sb = load2(wvT, "wvT", C)
            woT_sb = load2(woT, "woT", C)

            with tc.tile_pool(name="psum0", bufs=3, space="PSUM") as PS0, \
                 tc.tile_pool(name="psum1", bufs=4, space="PSUM") as PS1:
                # HAM warm-up: keep the PE busy while stats/DMA run so the
                # projections and attention run at 2.4 GHz from the start.
                warm_ps = PS0.tile([128, BLK], FP32, tag="warm", bufs=1)
                for _ in range(NWARM):
                    nc.tensor.matmul(
                        warm_ps, junk_bf[:, 0:128], junk_bf, start=True, stop=True
                    )

                # ---------------- phase 0: group stats -> folded affine ------
                # chunk 0 stats on the vector engine (bn_stats),
                # chunk 1 stats on the scalar engine (accum of x and x^2),
                # xb16 casts on gpsimd/scalar -> all three run in parallel.
                xb16 = []
                for j in range(NCH):
                    t = P.tile([128, N], BF16, tag=f"xb{j}", name=f"xb{j}")
                    xb16.append(t)
                nc.gpsimd.tensor_copy(out=xb16[0], in_=x_sb[0])

                t2 = []  # per chunk [128, 2]: col0 = mean, col1 = E[x^2]
                # chunk 0: bn_stats path
                stats = WK.tile([128, 8, 6], FP32, tag="bnstats")
                for s in range(8):
                    nc.vector.bn_stats(
                        out=stats[:, s, :], in_=x_sb[0][:, s * 512:(s + 1) * 512]
                    )
                mv = WK.tile([128, 2], FP32, tag="bnmv")
                nc.vector.bn_aggr(out=mv, in_=stats)
                t2_0 = WK.tile([128, 2], FP32, tag="chstat0")
                nc.vector.tensor_copy(out=t2_0[:, 0:1], in_=mv[:, 0:1])
                sq = WK.tile([128, 1], FP32, tag="chsq")
                nc.vector.tensor_mul(out=sq, in0=mv[:, 0:1], in1=mv[:, 0:1])
                nc.vector.tensor_add(out=t2_0[:, 1:2], in0=mv[:, 1:2], in1=sq)
                t2.append(t2_0)
                # chunk 1: scalar-engine accumulation path (also makes
                # xb16[1]); split into 4 pieces so it pipelines with the DMA.
                scratch = P.tile([128, N], BF16, tag="scratch")
                xsum_p = WK.tile([128, 8], FP32, tag="xsump")
                for p in range(4):
                    ps = slice(p * 1024, (p + 1) * 1024)
                    nc.scalar.activation(
                        out=xb16[1][:, ps], in_=x_sb[1][:, ps], func=AF.Copy,
                        accum_out=xsum_p[:, p:p + 1],
                    )
                    nc.scalar.activation(
                        out=scratch[:, ps], in_=x_sb[1][:, ps], func=AF.Square,
                        accum_out=xsum_p[:, 4 + p:5 + p],
                    )
                t2_1 = WK.tile([128, 2], FP32, tag="chstat1")
                sab = WK.tile([128, 4], FP32, tag="sab")
                nc.vector.tensor_add(
                    out=sab[:, 0:2], in0=xsum_p[:, 0:2], in1=xsum_p[:, 2:4]
                )
                nc.vector.tensor_add(
                    out=sab[:, 2:4], in0=xsum_p[:, 4:6], in1=xsum_p[:, 6:8]
                )
                nc.vector.tensor_add(
                    out=t2_1[:, 0:1], in0=sab[:, 0:1], in1=sab[:, 1:2]
                )
                nc.vector.tensor_add(
                    out=t2_1[:, 1:2], in0=sab[:, 2:3], in1=sab[:, 3:4]
                )
                nc.vector.tensor_scalar_mul(out=t2_1, in0=t2_1, scalar1=1.0 / N)
                t2.append(t2_1)

                gmr = []  # [16, 2] per chunk: col0 = group mean, col1 = rstd
                for j in range(NCH):
                    ps_g = PS0.tile([16, 2], FP32, tag="p0")
                    nc.tensor.matmul(ps_g, ind16_sb, t2[j], start=True, stop=True)
                    g2 = WK.tile([16, 2], FP32, tag="gstat")
                    nc.vector.tensor_copy(out=g2, in_=ps_g)
                    gsq = WK.tile([16, 1], FP32, tag="gsq")
                    nc.vector.tensor_mul(out=gsq, in0=g2[:, 0:1], in1=g2[:, 0:1])
                    gvar = WK.tile([16, 1], FP32, tag="gvar")
                    nc.vector.tensor_sub(out=gvar, in0=g2[:, 1:2], in1=gsq)
                    gsd = WK.tile([16, 1], FP32, tag="gsd")
                    nc.scalar.activation(
                        out=gsd, in_=gvar, func=AF.Sqrt, bias=eps16, scale=1.0
                    )
                    gm_r = WK.tile([16, 2], FP32, tag=f"gmr{j}")
                    nc.vector.tensor_copy(out=gm_r[:, 0:1], in_=g2[:, 0:1])
                    nc.vector.reciprocal(out=gm_r[:, 1:2], in_=gsd)
                    gmr.append(gm_r)

                fs_sb, fb_sb = [], []
                for j in range(NCH):
                    ps_bc = PS0.tile([128, 2], FP32, tag="p0")
                    nc.tensor.matmul(ps_bc, bcast16_sb, gmr[j], start=True, stop=True)
                    mbrb = WK.tile([128, 2], FP32, tag="mbrb")
                    nc.vector.tensor_copy(out=mbrb, in_=ps_bc)
                    fs = P.tile([128, 1], FP32, tag=f"fs{j}", name=f"fs{j}")
                    nc.vector.tensor_mul(out=fs, in0=gns_sb[j], in1=mbrb[:, 1:2])
                    tmp = WK.tile([128, 1], FP32, tag="fbt")
                    nc.vector.tensor_mul(out=tmp, in0=mbrb[:, 0:1], in1=fs)
                    fb = P.tile([128, 1], FP32, tag=f"fb{j}", name=f"fb{j}")
                    nc.vector.tensor_sub(out=fb, in0=gnb_sb[j], in1=tmp)
                    fs_sb.append(fs)
                    fb_sb.append(fb)

                # scaled bf16 weights: w'[c_in, c_out] = wT[c_in, c_out]*fs[c_in]
                wq_bf, wk_bf, wv_bf, wo_bf = [], [], [], []
                for name, src, dst in (
                    ("q", wqT_sb, wq_bf),
                    ("k", wkT_sb, wk_bf),
                    ("v", wvT_sb, wv_bf),
                ):
                    for j in range(NCH):
                        t = P.tile(
                            [128, C], BF16, tag=f"w{name}bf{j}", name=f"w{name}bf{j}"
                        )
                        nc.vector.tensor_scalar_mul(out=t, in0=src[j], scalar1=fs_sb[j])
                        dst.append(t)
                # fold the residual 1/sqrt(2) into the output weights
                for j in range(NCH):
                    t = P.tile([128, C], BF16, tag=f"wobf{j}", name=f"wobf{j}")
                    nc.vector.tensor_scalar_mul(
                        out=t, in0=woT_sb[j], scalar1=INV_SQRT2
                    )
                    wo_bf.append(t)

                # cQ = Wq @ fb + bq ; cV = Wv @ fb + bv ; bo_eff = Wo @ cV + bo
                cq_sb, cv_sb, boe_sb = [], [], []
                for name, wT, bias, dst in (
                    ("cq", wqT_sb, bq_sb, cq_sb),
                    ("cv", wvT_sb, bv_sb, cv_sb),
                ):
                    for o in range(NCH):
                        ps_c = PS0.tile([128, 1], FP32, tag="p0")
                        nc.tensor.matmul(
                            ps_c, wT[0][:, o * 128:(o + 1) * 128], fb_sb[0],
                            start=True, stop=False,
                        )
                        nc.tensor.matmul(
                            ps_c, wT[1][:, o * 128:(o + 1) * 128], fb_sb[1],
                            start=False, stop=True,
                        )
                        t = P.tile([128, 1], FP32, tag=f"{name}{o}", name=f"{name}{o}")
                        nc.vector.tensor_add(out=t, in0=ps_c, in1=bias[o])
                        dst.append(t)
                for o in range(NCH):
                    ps_c = PS0.tile([128, 1], FP32, tag="p0")
                    nc.tensor.matmul(
                        ps_c, woT_sb[0][:, o * 128:(o + 1) * 128], cv_sb[0],
                        start=True, stop=False,
                    )
                    nc.tensor.matmul(
                        ps_c, woT_sb[1][:, o * 128:(o + 1) * 128], cv_sb[1],
                        start=False, stop=True,
                    )
                    t = P.tile([128, 1], FP32, tag=f"boe{o}", name=f"boe{o}")
                    nc.vector.tensor_add(out=t, in0=ps_c, in1=bo_sb[o])
                    boe_sb.append(t)

                # ---------------- phase 1: projections ----------------
                q_sb = [P.tile([128, N], BF16, tag=f"q{o}", name=f"q{o}") for o in range(NCH)]
                k_sb = [P.tile([128, N], BF16, tag=f"k{o}", name=f"k{o}") for o in range(NCH)]
                vt_sb = P.tile([128, MT, C], BF16, tag="vt")

                for o in range(NCH):
                    for nb in range(NBLK):
                        cs = slice(nb * BLK, (nb + 1) * BLK)
                        ps_q = PS1.tile([128, BLK], FP32, tag="p1")
                        nc.tensor.matmul(
                            ps_q, wq_bf[0][:, o * 128:(o + 1) * 128], xb16[0][:, cs],
                            start=True, stop=False,
                        )
                        nc.tensor.matmul(
                            ps_q, wq_bf[1][:, o * 128:(o + 1) * 128], xb16[1][:, cs],
                            start=False, stop=True,
                        )
                        nc.scalar.activation(
                            out=q_sb[o][:, cs], in_=ps_q, func=AF.Identity,
                            bias=cq_sb[o], scale=1.0,
                        )
                        ps_k = PS1.tile([128, BLK], FP32, tag="p1")
                        nc.tensor.matmul(
                            ps_k, wk_bf[0][:, o * 128:(o + 1) * 128], xb16[0][:, cs],
                            start=True, stop=False,
                        )
                        nc.tensor.matmul(
                            ps_k, wk_bf[1][:, o * 128:(o + 1) * 128], xb16[1][:, cs],
                            start=False, stop=True,
                        )
                        nc.vector.tensor_copy(out=k_sb[o][:, cs], in_=ps_k)

                for i in range(MT):
                    ms = slice(i * 128, (i + 1) * 128)
                    ps_v = PS1.tile([128, C], FP32, tag="p1")
                    nc.tensor.matmul(
                        ps_v, xb16[0][:, ms], wv_bf[0], start=True, stop=False
                    )
                    nc.tensor.matmul(
                        ps_v, xb16[1][:, ms], wv_bf[1], start=False, stop=True
                    )
                    nc.vector.tensor_copy(out=vt_sb[:, i, :], in_=ps_v)

            # ---------------- phase 2: attention ----------------
            with tc.tile_pool(name="psum_s", bufs=1, space="PSUM") as PSS, \
                 tc.tile_pool(name="psum_av", bufs=1, space="PSUM") as PSAV, \
                 tc.tile_pool(name="psum_d", bufs=1, space="PSUM") as PSD, \
                 tc.tile_pool(name="psum_o", bufs=1, space="PSUM") as PSO:
                # explicit round-robin buffers: pool slot reuse is LIFO, which
                # would chain exp(i) behind the previous iteration's readers
                # (a ~430ns cross-engine handoff per iteration); fixed rotation
                # gives each consumer several iterations of slack instead.
                e_rot = [
                    P.tile([128, BLK], BF16, tag=f"e{r}", name=f"e{r}")
                    for r in range(6)
                ]
                s_rot = [
                    PSS.tile([128, BLK], FP32, tag=f"s{r}", name=f"s{r}")
                    for r in range(4)
                ]
                # deferred epilogue stages of the previous block, dispatched
                # a few iterations into the current block's PE stream so the
                # reciprocal/DVE handoffs never stall the tensor engine.
                pending = {}

                for nb in range(NBLK):
                    cs = slice(nb * BLK, (nb + 1) * BLK)
                    ps_av = [
                        PSAV.tile([128, BLK], FP32, tag=f"av{o}", name=f"av{o}")
                        for o in range(NCH)
                    ]
                    ps_d = PSD.tile([1, BLK], FP32, tag="d")
                    s_tiles = {}

                    def emit_s(i, cs=cs, s_tiles=s_tiles, nb=nb):
                        ms = slice(i * 128, (i + 1) * 128)
                        ps_s = s_rot[(nb * MT + i) % 4]
                        with nc.named_scope("smm"):
                            nc.tensor.matmul(
                                ps_s, k_sb[0][:, ms], q_sb[0][:, cs],
                                start=True, stop=False,
                            )
                            nc.tensor.matmul(
                                ps_s, k_sb[1][:, ms], q_sb[1][:, cs],
                                start=False, stop=True,
                            )
                        s_tiles[i] = ps_s

                    for i in range(3):
                        emit_s(i)
                    for i in range(MT):
                        if i + 3 < MT:
                            emit_s(i + 3)
                        e_i = e_rot[(nb * MT + i) % 6]
                        with nc.named_scope("exp"):
                            nc.scalar.activation(
                                out=e_i, in_=s_tiles.pop(i), func=AF.Exp, scale=SCALE
                            )
                        with nc.named_scope("dmm"):
                            nc.tensor.matmul(
                                ps_d, ones128_bf, e_i, start=(i == 0), stop=(i == MT - 1)
                            )
                        with nc.named_scope("avmm"):
                            for o in range(NCH):
                                nc.tensor.matmul(
                                    ps_av[o], vt_sb[:, i, o * 128:(o + 1) * 128], e_i,
                                    start=(i == 0), stop=(i == MT - 1),
                                )
                        if i in pending:
                            pending.pop(i)()

                    # part A (vector engine only): free the psum accumulators
                    d_sb = WK.tile([1, BLK], FP32, tag="dsb")
                    nc.vector.tensor_copy(out=d_sb, in_=ps_d)
                    av_sb = []
                    for o in range(NCH):
                        t = WK.tile([128, BLK], BF16, tag=f"avsb{o}", name=f"avsb{o}")
                        nc.vector.tensor_copy(out=t, in_=ps_av[o])
                        av_sb.append(t)
                    recip = WK.tile([1, BLK], FP32, tag="recip")
                    nc.vector.reciprocal(out=recip, in_=d_sb)

                    rb_box = {}

                    def stage_rb(recip=recip, rb_box=rb_box):
                        ps_rb = PSO.tile([128, BLK], FP32, tag="o", name="rb")
                        nc.tensor.matmul(
                            ps_rb, ones1_f32, recip, start=True, stop=True
                        )
                        rb_sb = WK.tile([128, BLK], FP32, tag="rbsb")
                        nc.vector.tensor_copy(out=rb_sb, in_=ps_rb)
                        rb_box["rb"] = rb_sb

                    def stage_o(o, cs=cs, av_sb=av_sb, rb_box=rb_box):
                        ps_o = PSO.tile([128, BLK], FP32, tag="o", name=f"o{o}")
                        nc.tensor.matmul(
                            ps_o, wo_bf[0][:, o * 128:(o + 1) * 128], av_sb[0],
                            start=True, stop=False,
                        )
                        nc.tensor.matmul(
                            ps_o, wo_bf[1][:, o * 128:(o + 1) * 128], av_sb[1],
                            start=False, stop=True,
                        )
                        # y = x/sqrt2 + bo_eff/sqrt2 + (wo/sqrt2 @ AV)/denom
                        xb_t = WK.tile([128, BLK], FP32, tag="xbt")
                        nc.vector.tensor_scalar(
                            out=xb_t, in0=x_sb[o][:, cs],
                            scalar1=boe_sb[o], scalar2=INV_SQRT2,
                            op0=ALU.add, op1=ALU.mult,
                        )
                        t_t = WK.tile([128, BLK], FP32, tag="tt2")
                        nc.vector.tensor_tensor(
                            out=t_t, in0=ps_o, in1=rb_box["rb"], op=ALU.mult
                        )
                        y_t = WK.tile([128, BLK], FP32, tag="yt")
                        nc.vector.tensor_add(out=y_t, in0=t_t, in1=xb_t)
                        nc.sync.dma_start(
                            out=y[o * 128:(o + 1) * 128, cs], in_=y_t
                        )

                    if nb + 1 < NBLK:
                        pending = {
                            5: stage_rb,
                            8: lambda: stage_o(0),
                            11: lambda: stage_o(1),
                        }
                    else:
                        stage_rb()
                        stage_o(0)
                        stage_o(1)

    nc.compile()
    return nc


_PROGRAM = None


def _get_program():
    global _PROGRAM
    if _PROGRAM is None:
        _PROGRAM = build_program()
    return _PROGRAM


def make_in_maps(inputs):
    x = np.ascontiguousarray(np.asarray(inputs["x"], dtype=np.float32))
    shared = {
        "wqT": np.ascontiguousarray(np.asarray(inputs["w_q"], np.float32).T),
        "wkT": np.ascontiguousarray(np.asarray(inputs["w_k"], np.float32).T),
        "wvT": np.ascontiguousarray(np.asarray(inputs["w_v"], np.float32).T),
        "woT": np.ascontiguousarray(np.asarray(inputs["w_o"], np.float32).T),
        "bq": np.asarray(inputs["b_q"], np.float32).reshape(C, 1).copy(),
        "bv": np.asarray(inputs["b_v"], np.float32).reshape(C, 1).copy(),
        "bo": np.asarray(inputs["b_o"], np.float32).reshape(C, 1).copy(),
        "gns": np.asarray(inputs["gn_scale"], np.float32).reshape(C, 1).copy(),
        "gnb": np.asarray(inputs["gn_bias"], np.float32).reshape(C, 1).copy(),
        "ind16": (
            (np.arange(128)[:, None] // GD == np.arange(16)[None, :]) / GD
        ).astype(np.float32),
        "bcast16": (
            np.arange(16)[:, None] == np.arange(128)[None, :] // GD
        ).astype(np.float32),
    }
    in_maps = []
    for i in range(NCORES):
        m = dict(shared)
        m["x"] = np.ascontiguousarray(x[i].reshape(C, N))
        in_maps.append(m)
    return in_maps


def run(inputs, trace=False, trace_cores=None):
    nc = _get_program()
    in_maps = make_in_maps(inputs)
    res = bass_utils.run_bass_kernel_spmd(
        nc, in_maps, core_ids=list(range(NCORES)), trace=trace,
        trace_cores=trace_cores,
    )
    out = np.stack(
        [res.results[i]["y"].reshape(C, H, W) for i in range(NCORES)]
    ).astype(np.float32)
    return out, res


def kernel(**inputs) -> np.ndarray:
    out, _ = run(inputs, trace=False)
    return out


# revision 8
# speedup vs baseline: 1.0022x; 1.0022x over previous
"""Trainium2 Bass kernel for an AttnBlock++ (GroupNorm -> QKV 1x1 conv ->
full softmax attention over HW tokens -> output projection -> residual/sqrt(2)).

Sharding: data-parallel over batch B=8 across the 8 NeuronCores; attention is
fully independent per batch element, so each core processes one [C, H*W]
feature map with no collectives.

Per-core algorithm (C=256, N=H*W=4096, 32 groups):
  - GroupNorm is folded into the QKV weights: hn = fs*x + fb (per-channel
    affine from group stats), so q/k/v = (W*diag(fs)) @ x + const. The K-side
    constant drops out of softmax exactly (shift invariance); the V-side
    constant is folded into the output-projection bias (attention rows sum
    to 1); only the Q-side constant is applied.
  - Scores are computed transposed (S_T[m, n], keys on partitions) so no
    transposes are ever needed: exp on the scalar engine, softmax denominator
    via a ones-vector matmul on the tensor engine (partition-axis sum), and
    att@V with natural [c, n] output. Division by the denominator commutes
    with the output projection and is applied at the very end.
  - 1/sqrt(2) of the residual is folded into the output weights and biases.
"""

import math

import numpy as np

import concourse.bacc as bacc
import concourse.tile as tile
from concourse.tile import add_dep_helper
from concourse import mybir
from concourse import bass_utils

B, C, H, W = 8, 256, 64, 64
N = H * W  # 4096
G = 32  # groups
GD = C // G  # 8 channels per group
EPS = 1e-6
NCORES = 8
NCH = 2  # channel chunks of 128
NBLK = 8  # query blocks
BLK = 512  # queries per block
MT = 32  # key tiles of 128
SCALE = float(C) ** -0.5  # 1/16
INV_SQRT2 = float(1.0 / math.sqrt(2.0))
NWARM = 56  # HAM warm-up matmuls issued while phase 0 runs

FP32 = mybir.dt.float32
BF16 = mybir.dt.bfloat16
AF = mybir.ActivationFunctionType
ALU = mybir.AluOpType


def build_program():
    nc = bacc.Bacc("TRN2", target_bir_lowering=False, debug=False)

    x = nc.dram_tensor("x", [C, N], FP32, kind="ExternalInput").ap()
    wqT = nc.dram_tensor("wqT", [C, C], FP32, kind="ExternalInput").ap()
    wkT = nc.dram_tensor("wkT", [C, C], FP32, kind="ExternalInput").ap()
    wvT = nc.dram_tensor("wvT", [C, C], FP32, kind="ExternalInput").ap()
    woT = nc.dram_tensor("woT", [C, C], FP32, kind="ExternalInput").ap()
    bq = nc.dram_tensor("bq", [C, 1], FP32, kind="ExternalInput").ap()
    bv = nc.dram_tensor("bv", [C, 1], FP32, kind="ExternalInput").ap()
    bo = nc.dram_tensor("bo", [C, 1], FP32, kind="ExternalInput").ap()
    gns = nc.dram_tensor("gns", [C, 1], FP32, kind="ExternalInput").ap()
    gnb = nc.dram_tensor("gnb", [C, 1], FP32, kind="ExternalInput").ap()
    # ind16[c, g] = 1/8 if c//8 == g else 0 (group-average over channels)
    ind16 = nc.dram_tensor("ind16", [128, 16], FP32, kind="ExternalInput").ap()
    # bcast16[g, c] = 1 if c//8 == g else 0 (broadcast group value to channels)
    bcast16 = nc.dram_tensor("bcast16", [16, 128], FP32, kind="ExternalInput").ap()
    y = nc.dram_tensor("y", [C, N], FP32, kind="ExternalOutput").ap()

    with tile.TileContext(nc) as tc:
        with (
            tc.tile_pool(name="persist", bufs=1) as P,
            tc.tile_pool(name="work", bufs=3) as WK,
        ):
            # ---------------- constants + HAM warm-up ----------------
            junk_bf = P.tile([128, BLK], BF16, tag="junk")
            nc.gpsimd.memset(junk_bf, 0.0)
            ones128_bf = P.tile([128, 1], BF16, tag="ones128")
            nc.vector.memset(ones128_bf, 1.0)
            ones1_f32 = P.tile([1, 128], FP32, tag="ones1")
            nc.vector.memset(ones1_f32, 1.0)
            eps16 = P.tile([16, 1], FP32, tag="eps16")
            nc.vector.memset(eps16, EPS)

            # ---------------- load inputs (x first: stats critical path) ----
            x_sb = []
            for j in range(NCH):
                t = P.tile([128, N], FP32, tag=f"x{j}", name=f"x{j}")
                x_sb.append(t)
            for j in range(NCH):
                for p in range(4):
                    cs = slice(p * 1024, (p + 1) * 1024)
                    nc.sync.dma_start(
                        out=x_sb[j][:, cs], in_=x[j * 128:(j + 1) * 128, cs]
                    )

            def load2(ap, name, width):
                ts = []
                for j in range(NCH):
                    t = P.tile([128, width], FP32, tag=f"{name}{j}", name=f"{name}{j}")
                    nc.sync.dma_start(out=t, in_=ap[j * 128:(j + 1) * 128, :])
                    ts.append(t)
                return ts

            ind16_sb = P.tile([128, 16], FP32, tag="ind16")
            nc.sync.dma_start(out=ind16_sb, in_=ind16)
            bcast16_sb = P.tile([16, 128], FP32, tag="bcast16")
            nc.sync.dma_start(out=bcast16_sb, in_=bcast16)
            gns_sb = load2(gns, "gns", 1)
            gnb_sb = load2(gnb, "gnb", 1)
            bq_sb = load2(bq, "bq", 1)
            bv_sb = load2(bv, "bv", 1)
            bo_sb = load2(bo, "bo", 1)
            wqT_sb = load2(wqT, "wqT", C)
            wkT_sb = load2(wkT, "wkT", C)
            wvT_sb = load2(wvT, "wvT", C)
            woT_sb = load2(woT, "woT", C)

            with tc.tile_pool(name="psum0", bufs=3, space="PSUM") as PS0, \
                 tc.tile_pool(name="psum1", bufs=4, space="PSUM") as PS1:
                # HAM warm-up: keep the PE busy while stats/DMA run so the
                # projections and attention run at 2.4 GHz from the start.
                warm_ps = PS0.tile([128, BLK], FP32, tag="warm", bufs=1)
                for _ in range(NWARM):
                    nc.tensor.matmul(
                        warm_ps, junk_bf[:, 0:128], junk_bf, start=True, stop=True
                    )

                # ---------------- phase 0: group stats -> folded affine ------
                # chunk 0 stats on the vector engine (bn_stats),
                # chunk 1 stats on the scalar engine (accum of x and x^2),
                # xb16 casts on gpsimd/scalar -> all three run in parallel.
                xb16 = []
                for j in range(NCH):
                    t = P.tile([128, N], BF16, tag=f"xb{j}", name=f"xb{j}")
                    xb16.append(t)
                nc.gpsimd.tensor_copy(out=xb16[0], in_=x_sb[0])

                t2 = []  # per chunk [128, 2]: col0 = mean, col1 = E[x^2]
                # chunk 0: bn_stats path
                stats = WK.tile([128, 8, 6], FP32, tag="bnstats")
                for s in range(8):
                    nc.vector.bn_stats(
                        out=stats[:, s, :], in_=x_sb[0][:, s * 512:(s + 1) * 512]
                    )
                mv = WK.tile([128, 2], FP32, tag="bnmv")
                nc.vector.bn_aggr(out=mv, in_=stats)
                t2_0 = WK.tile([128, 2], FP32, tag="chstat0")
                nc.vector.tensor_copy(out=t2_0[:, 0:1], in_=mv[:, 0:1])
                sq = WK.tile([128, 1], FP32, tag="chsq")
                nc.vector.tensor_mul(out=sq, in0=mv[:, 0:1], in1=mv[:, 0:1])
                nc.vector.tensor_add(out=t2_0[:, 1:2], in0=mv[:, 1:2], in1=sq)
                t2.append(t2_0)
                # chunk 1: scalar-engine accumulation path (also makes
                # xb16[1]); split into 4 pieces so it pipelines with the DMA.
                scratch = P.tile([128, N], BF16, tag="scratch")
                xsum_p = WK.tile([128, 8], FP32, tag="xsump")
                for p in range(4):
                    ps = slice(p * 1024, (p + 1) * 1024)
                    nc.scalar.activation(
                        out=xb16[1][:, ps], in_=x_sb[1][:, ps], func=AF.Copy,
                        accum_out=xsum_p[:, p:p + 1],
                    )
                    nc.scalar.activation(
                        out=scratch[:, ps], in_=x_sb[1][:, ps], func=AF.Square,
                        accum_out=xsum_p[:, 4 + p:5 + p],
                    )
                t2_1 = WK.tile([128, 2], FP32, tag="chstat1")
                sab = WK.tile([128, 4], FP32, tag="sab")
                nc.vector.tensor_add(
                    out=sab[:, 0:2], in0=xsum_p[:, 0:2], in1=xsum_p[:, 2:4]
                )
                nc.vector.tensor_add(
                    out=sab[:, 2:4], in0=xsum_p[:, 4:6], in1=xsum_p[:, 6:8]
                )
                nc.vector.tensor_add(
                    out=t2_1[:, 0:1], in0=sab[:, 0:1], in1=sab[:, 1:2]
                )
                nc.vector.tensor_add(
                    out=t2_1[:, 1:2], in0=sab[:, 2:3], in1=sab[:, 3:4]
                )
                nc.vector.tensor_scalar_mul(out=t2_1, in0=t2_1, scalar1=1.0 / N)
                t2.append(t2_1)

                gmr = []  # [16, 2] per chunk: col0 = group mean, col1 = rstd
                for j in range(NCH):
                    ps_g = PS0.tile([16, 2], FP32, tag="p0")
                    nc.tensor.matmul(ps_g, ind16_sb, t2[j], start=True, stop=True)
                    g2 = WK.tile([16, 2], FP32, tag="gstat")
                    nc.vector.tensor_copy(out=g2, in_=ps_g)
                    gsq = WK.tile([16, 1], FP32, tag="gsq")
                    nc.vector.tensor_mul(out=gsq, in0=g2[:, 0:1], in1=g2[:, 0:1])
                    gvar = WK.tile([16, 1], FP32, tag="gvar")
                    nc.vector.tensor_sub(out=gvar, in0=g2[:, 1:2], in1=gsq)
                    gsd = WK.tile([16, 1], FP32, tag="gsd")
                    nc.scalar.activation(
                        out=gsd, in_=gvar, func=AF.Sqrt, bias=eps16, scale=1.0
                    )
                    gm_r = WK.tile([16, 2], FP32, tag=f"gmr{j}")
                    nc.vector.tensor_copy(out=gm_r[:, 0:1], in_=g2[:, 0:1])
                    nc.vector.reciprocal(out=gm_r[:, 1:2], in_=gsd)
                    gmr.append(gm_r)

                fs_sb, fb_sb = [], []
                for j in range(NCH):
                    ps_bc = PS0.tile([128, 2], FP32, tag="p0")
                    nc.tensor.matmul(ps_bc, bcast16_sb, gmr[j], start=True, stop=True)
                    mbrb = WK.tile([128, 2], FP32, tag="mbrb")
                    nc.vector.tensor_copy(out=mbrb, in_=ps_bc)
                    fs = P.tile([128, 1], FP32, tag=f"fs{j}", name=f"fs{j}")
                    nc.vector.tensor_mul(out=fs, in0=gns_sb[j], in1=mbrb[:, 1:2])
                    tmp = WK.tile([128, 1], FP32, tag="fbt")
                    nc.vector.tensor_mul(out=tmp, in0=mbrb[:, 0:1], in1=fs)
                    fb = P.tile([128, 1], FP32, tag=f"fb{j}", name=f"fb{j}")
                    nc.vector.tensor_sub(out=fb, in0=gnb_sb[j], in1=tmp)
                    fs_sb.append(fs)
                    fb_sb.append(fb)

                # scaled bf16 weights: w'[c_in, c_out] = wT[c_in, c_out]*fs[c_in]
                wq_bf, wk_bf, wv_bf, wo_bf = [], [], [], []
                for name, src, dst in (
                    ("q", wqT_sb, wq_bf),
                    ("k", wkT_sb, wk_bf),
                    ("v", wvT_sb, wv_bf),
                ):
                    for j in range(NCH):
                        t = P.tile(
                            [128, C], BF16, tag=f"w{name}bf{j}", name=f"w{name}bf{j}"
                        )
                        nc.vector.tensor_scalar_mul(out=t, in0=src[j], scalar1=fs_sb[j])
                        dst.append(t)
                # fold the residual 1/sqrt(2) into the output weights
                for j in range(NCH):
                    t = P.tile([128, C], BF16, tag=f"wobf{j}", name=f"wobf{j}")
                    nc.vector.tensor_scalar_mul(
                        out=t, in0=woT_sb[j], scalar1=INV_SQRT2
                    )
                    wo_bf.append(t)

                # cQ = Wq @ fb + bq ; cV = Wv @ fb + bv ; bo_eff = Wo @ cV + bo
                cq_sb, cv_sb, boe_sb = [], [], []
                for name, wT, bias, dst in (
                    ("cq", wqT_sb, bq_sb, cq_sb),
                    ("cv", wvT_sb, bv_sb, cv_sb),
                ):
                    for o in range(NCH):
                        ps_c = PS0.tile([128, 1], FP32, tag="p0")
                        nc.tensor.matmul(
                            ps_c, wT[0][:, o * 128:(o + 1) * 128], fb_sb[0],
                            start=True, stop=False,
                        )
                        nc.tensor.matmul(
                            ps_c, wT[1][:, o * 128:(o + 1) * 128], fb_sb[1],
                            start=False, stop=True,
                        )
                        t = P.tile([128, 1], FP32, tag=f"{name}{o}", name=f"{name}{o}")
                        nc.vector.tensor_add(out=t, in0=ps_c, in1=bias[o])
                        dst.append(t)
                for o in range(NCH):
                    ps_c = PS0.tile([128, 1], FP32, tag="p0")
                    nc.tensor.matmul(
                        ps_c, woT_sb[0][:, o * 128:(o + 1) * 128], cv_sb[0],
                        start=True, stop=False,
                    )
                    nc.tensor.matmul(
                        ps_c, woT_sb[1][:, o * 128:(o + 1) * 128], cv_sb[1],
                        start=False, stop=True,
                    )
                    t = P.tile([128, 1], FP32, tag=f"boe{o}", name=f"boe{o}")
                    nc.vector.tensor_add(out=t, in0=ps_c, in1=bo_sb[o])
                    boe_sb.append(t)

                # ---------------- phase 1: projections ----------------
                q_sb = [P.tile([128, N], BF16, tag=f"q{o}", name=f"q{o}") for o in range(NCH)]
                k_sb = [P.tile([128, N], BF16, tag=f"k{o}", name=f"k{o}") for o in range(NCH)]
                vt_sb = P.tile([128, MT, C], BF16, tag="vt")

                for o in range(NCH):
                    for nb in range(NBLK):
                        cs = slice(nb * BLK, (nb + 1) * BLK)
                        ps_q = PS1.tile([128, BLK], FP32, tag="p1")
                        nc.tensor.matmul(
                            ps_q, wq_bf[0][:, o * 128:(o + 1) * 128], xb16[0][:, cs],
                            start=True, stop=False,
                        )
                        nc.tensor.matmul(
                            ps_q, wq_bf[1][:, o * 128:(o + 1) * 128], xb16[1][:, cs],
                            start=False, stop=True,
                        )
                        nc.scalar.activation(
                            out=q_sb[o][:, cs], in_=ps_q, func=AF.Identity,
                            bias=cq_sb[o], scale=1.0,
                        )
                        ps_k = PS1.tile([128, BLK], FP32, tag="p1")
                        nc.tensor.matmul(
                            ps_k, wk_bf[0][:, o * 128:(o + 1) * 128], xb16[0][:, cs],
                            start=True, stop=False,
                        )
                        nc.tensor.matmul(
                            ps_k, wk_bf[1][:, o * 128:(o + 1) * 128], xb16[1][:, cs],
                            start=False, stop=True,
                        )
                        nc.vector.tensor_copy(out=k_sb[o][:, cs], in_=ps_k)

                for i in range(MT):
                    ms = slice(i * 128, (i + 1) * 128)
                    ps_v = PS1.tile([128, C], FP32, tag="p1")
                    nc.tensor.matmul(
                        ps_v, xb16[0][:, ms], wv_bf[0], start=True, stop=False
                    )
                    nc.tensor.matmul(
                        ps_v, xb16[1][:, ms], wv_bf[1], start=False, stop=True
                    )
                    nc.vector.tensor_copy(out=vt_sb[:, i, :], in_=ps_v)

            # ---------------- phase 2: attention ----------------
            with tc.tile_pool(name="psum_s", bufs=1, space="PSUM") as PSS, \
                 tc.tile_pool(name="psum_av", bufs=1, space="PSUM") as PSAV, \
                 tc.tile_pool(name="psum_d", bufs=1, space="PSUM") as PSD, \
                 tc.tile_pool(name="psum_o", bufs=1, space="PSUM") as PSO:
                # explicit round-robin buffers: pool slot reuse is LIFO, which
                # would chain exp(i) behind the previous iteration's readers
                # (a ~430ns cross-engine handoff per iteration); fixed rotation
                # gives each consumer several iterations of slack instead.
                e_rot = [
                    P.tile([128, BLK], BF16, tag=f"e{r}", name=f"e{r}")
                    for r in range(6)
                ]
                s_rot = [
                    PSS.tile([128, BLK], FP32, tag=f"s{r}", name=f"s{r}")
                    for r in range(4)
                ]
                # deferred epilogue stages of the previous block, dispatched
                # a few iterations into the current block's PE stream so the
                # reciprocal/DVE handoffs never stall the tensor engine.
                pending = {}
                # forced pipeline depth: the scheduler otherwise compresses the
                # S->exp->d chain to one iteration of lead, exposing the
                # ~0.8us exp handoff on every iteration. Order-only deps pin
                # d(k) after S1(k+3) so exp(k) always has ~12 matmuls of cover.
                s1_handles = []
                d_handles = []
                wired = [0]

                def wire_deps():
                    while (wired[0] < len(d_handles)
                           and wired[0] + 3 < len(s1_handles)):
                        k = wired[0]
                        add_dep_helper(
                            d_handles[k].ins, s1_handles[k + 3].ins,
                            sync=False, reason="exp pipeline depth",
                        )
                        wired[0] += 1

                for nb in range(NBLK):
                    cs = slice(nb * BLK, (nb + 1) * BLK)
                    ps_av = [
                        PSAV.tile([128, BLK], FP32, tag=f"av{o}", name=f"av{o}")
                        for o in range(NCH)
                    ]
                    ps_d = PSD.tile([1, BLK], FP32, tag="d")
                    s_tiles = {}

                    def emit_s(i, cs=cs, s_tiles=s_tiles, nb=nb):
                        ms = slice(i * 128, (i + 1) * 128)
                        ps_s = s_rot[(nb * MT + i) % 4]
                        with nc.named_scope("smm"):
                            nc.tensor.matmul(
                                ps_s, k_sb[0][:, ms], q_sb[0][:, cs],
                                start=True, stop=False,
                            )
                            s1 = nc.tensor.matmul(
                                ps_s, k_sb[1][:, ms], q_sb[1][:, cs],
                                start=False, stop=True,
                            )
                        s1_handles.append(s1)
                        s_tiles[i] = ps_s
                        wire_deps()

                    for i in range(3):
                        emit_s(i)
                    for i in range(MT):
                        if i + 3 < MT:
                            emit_s(i + 3)
                        e_i = e_rot[(nb * MT + i) % 6]
                        with nc.named_scope("exp"):
                            nc.scalar.activation(
                                out=e_i, in_=s_tiles.pop(i), func=AF.Exp, scale=SCALE
                            )
                        with nc.named_scope("dmm"):
                            dmm = nc.tensor.matmul(
                                ps_d, ones128_bf, e_i, start=(i == 0), stop=(i == MT - 1)
                            )
                        d_handles.append(dmm)
                        wire_deps()
                        with nc.named_scope("avmm"):
                            for o in range(NCH):
                                nc.tensor.matmul(
                                    ps_av[o], vt_sb[:, i, o * 128:(o + 1) * 128], e_i,
                                    start=(i == 0), stop=(i == MT - 1),
                                )
                        if i in pending:
                            pending.pop(i)()

                    # part A (vector engine only): free the psum accumulators
                    d_sb = WK.tile([1, BLK], FP32, tag="dsb")
                    nc.vector.tensor_copy(out=d_sb, in_=ps_d)
                    av_sb = []
                    for o in range(NCH):
                        t = WK.tile([128, BLK], BF16, tag=f"avsb{o}", name=f"avsb{o}")
                        nc.vector.tensor_copy(out=t, in_=ps_av[o])
                        av_sb.append(t)
                    recip = WK.tile([1, BLK], FP32, tag="recip")
                    nc.vector.reciprocal(out=recip, in_=d_sb)

                    rb_box = {}

                    def stage_rb(recip=recip, rb_box=rb_box):
                        ps_rb = PSO.tile([128, BLK], FP32, tag="o", name="rb")
                        nc.tensor.matmul(
                            ps_rb, ones1_f32, recip, start=True, stop=True
                        )
                        rb_sb = WK.tile([128, BLK], FP32, tag="rbsb")
                        nc.vector.tensor_copy(out=rb_sb, in_=ps_rb)
                        rb_box["rb"] = rb_sb

                    def stage_o(o, cs=cs, av_sb=av_sb, rb_box=rb_box):
                        ps_o = PSO.tile([128, BLK], FP32, tag="o", name=f"o{o}")
                        nc.tensor.matmul(
                            ps_o, wo_bf[0][:, o * 128:(o + 1) * 128], av_sb[0],
                            start=True, stop=False,
                        )
                        nc.tensor.matmul(
                            ps_o, wo_bf[1][:, o * 128:(o + 1) * 128], av_sb[1],
                            start=False, stop=True,
                        )
                        # y = x/sqrt2 + bo_eff/sqrt2 + (wo/sqrt2 @ AV)/denom
                        xb_t = WK.tile([128, BLK], FP32, tag="xbt")
                        nc.vector.tensor_scalar(
                            out=xb_t, in0=x_sb[o][:, cs],
                            scalar1=boe_sb[o], scalar2=INV_SQRT2,
                            op0=ALU.add, op1=ALU.mult,
                        )
                        t_t = WK.tile([128, BLK], FP32, tag="tt2")
                        nc.vector.tensor_tensor(
                            out=t_t, in0=ps_o, in1=rb_box["rb"], op=ALU.mult
                        )
                        y_t = WK.tile([128, BLK], FP32, tag="yt")
                        nc.vector.tensor_add(out=y_t, in0=t_t, in1=xb_t)
                        nc.sync.dma_start(
                            out=y[o * 128:(o + 1) * 128, cs], in_=y_t
                        )

                    if nb + 1 < NBLK:
                        pending = {
                            5: stage_rb,
                            8: lambda: stage_o(0),
                            11: lambda: stage_o(1),
                        }
                    else:
                        stage_rb()
                        stage_o(0)
                        stage_o(1)

    nc.compile()
    return nc


_PROGRAM = None


def _get_program():
    global _PROGRAM
    if _PROGRAM is None:
        _PROGRAM = build_program()
    return _PROGRAM


def make_in_maps(inputs):
    x = np.ascontiguousarray(np.asarray(inputs["x"], dtype=np.float32))
    shared = {
        "wqT": np.ascontiguousarray(np.asarray(inputs["w_q"], np.float32).T),
        "wkT": np.ascontiguousarray(np.asarray(inputs["w_k"], np.float32).T),
        "wvT": np.ascontiguousarray(np.asarray(inputs["w_v"], np.float32).T),
        "woT": np.ascontiguousarray(np.asarray(inputs["w_o"], np.float32).T),
        "bq": np.asarray(inputs["b_q"], np.float32).reshape(C, 1).copy(),
        "bv": np.asarray(inputs["b_v"], np.float32).reshape(C, 1).copy(),
        "bo": np.asarray(inputs["b_o"], np.float32).reshape(C, 1).copy(),
        "gns": np.asarray(inputs["gn_scale"], np.float32).reshape(C, 1).copy(),
        "gnb": np.asarray(inputs["gn_bias"], np.float32).reshape(C, 1).copy(),
        "ind16": (
            (np.arange(128)[:, None] // GD == np.arange(16)[None, :]) / GD
        ).astype(np.float32),
        "bcast16": (
            np.arange(16)[:, None] == np.arange(128)[None, :] // GD
        ).astype(np.float32),
    }
    in_maps = []
    for i in range(NCORES):
        m = dict(shared)
        m["x"] = np.ascontiguousarray(x[i].reshape(C, N))
        in_maps.append(m)
    return in_maps


def run(inputs, trace=False, trace_cores=None):
    nc = _get_program()
    in_maps = make_in_maps(inputs)
    res = bass_utils.run_bass_kernel_spmd(
        nc, in_maps, core_ids=list(range(NCORES)), trace=trace,
        trace_cores=trace_cores,
    )
    out = np.stack(
        [res.results[i]["y"].reshape(C, H, W) for i in range(NCORES)]
    ).astype(np.float32)
    return out, res


def kernel(**inputs) -> np.ndarray:
    out, _ = run(inputs, trace=False)
    return out
